# revision 32
# baseline (speedup 1.0000x reference)
"""Cross-attention kernel for Trainium2, 8 NeuronCores.

Sharding (data + head parallel, per the problem's sharding hint):
  core c in 0..7 -> batch b = c // 4, head-pair hp = c % 4.
  Each core computes attention for its batch with 2 of the 8 heads
  (a 128-wide slice of the 512 hidden features), then the partial
  out-projection  attn_out_slice @ Wo[slice, :].  The host sums the 4
  partials per batch (the "all-reduce") and adds bo.

Device-side dataflow per core (all matmuls bf16, feature-major):
  All inputs land in SBUF via a few ~1MB DMAs spread over the three
  DMA rings (sync/SP HWDGE, scalar/ACT HWDGE, gpsimd SWDGE) so no ring
  serializes the kernel.
  qT[128, N] = Wq_sl.T @ xT          (contraction over D=1024 in 8 chunks)
  kT[128, M] = Wk_sl.T @ cT ; vT likewise
  Vall[m, 130] = DMA-xbar transpose of vT per m-chunk, with ones
      columns at 0 and 129: PV stationary for h0 = cols 0:65 (den in
      output row 0), for h1 = cols 65:130 (den in output row 64).
  per n-chunk s (512 cols), per m-chunk mc (128 rows), software-
  pipelined so ScalarE (exp) never idles:
     St[m 128, 1024] = [kT_h0_mc.T @ qT_h0_s | kT_h1_mc.T @ qT_h1_s]
         (concurrent matmuls on PE row-groups 0-63 / 64-127)
     Pt = exp(St * 1/8)               (ScalarE -- the bottleneck engine)
     Oaug_h[65, 512] += Vall_h_mc.T @ Pt_h               (PSUM accum)
  fin: denominators from the ones rows; reciprocal repartitioned via a
  DRAM bounce; OT[h*64:, s] = O_h / den_h.
  outproj: out[n 128, 1024] = OT_ntile.T @ Wo_sl, staged in SBUF,
  written back as two 1MB DMAs per n-chunk (bias bo added on host).
"""

import numpy as np

import concourse.bass as bass
import concourse.tile as tile
from concourse import bacc, mybir
from concourse.masks import make_identity

F32 = mybir.dt.float32
BF16 = mybir.dt.bfloat16

D = 1024      # model dim (contraction for projections)
SEQ = 2048    # n == m
F = 128       # features per core (2 heads x 64)
DH = 64       # head dim
NS = SEQ // 512   # 4 n-chunks of 512
NK = D // 128     # 8 contraction chunks
NM = SEQ // 128   # 16 m-chunks of 128
VP = 132          # Vall row stride (130 used + 2 pad)
SCALE = DH ** -0.5
EXPF = mybir.ActivationFunctionType.Exp


def build_nc():
    nc = bacc.Bacc("TRN2", target_bir_lowering=False, debug=False)

    # x/ctx arrive host-blocked: block j (512 seq cols) is [128, NK*512]
    # with fully contiguous per-partition runs, so each block DMA needs
    # only 128 8KB-contiguous descriptors.
    xT_d = nc.dram_tensor("xT", [NS * 128, NK * 512], BF16, kind="ExternalInput")
    cT_d = nc.dram_tensor("cT", [NS * 128, NK * 512], BF16, kind="ExternalInput")
    # wq/wk/wv arrive pre-swizzled by the host: [128, NK*128] where
    # column block k holds W[k*128:(k+1)*128, :].T-chunk laid contiguously.
    wq_d = nc.dram_tensor("wq", [128, NK * 128], BF16, kind="ExternalInput")
    wk_d = nc.dram_tensor("wk", [128, NK * 128], BF16, kind="ExternalInput")
    wv_d = nc.dram_tensor("wv", [128, NK * 128], BF16, kind="ExternalInput")
    wo_d = nc.dram_tensor("wo", [F, D], BF16, kind="ExternalInput")
    out_d = nc.dram_tensor("out_p", [SEQ, D], F32, kind="ExternalOutput")

    with tile.TileContext(nc) as tc:
        _emit(tc, nc, xT_d, cT_d, wq_d, wk_d, wv_d, wo_d, out_d)
    nc.compile()
    return nc


def _emit(tc, nc, xT_d, cT_d, wq_d, wk_d, wv_d, wo_d, out_d, dbg=None):
    from contextlib import ExitStack

    ctx = ExitStack()
    wpool = ctx.enter_context(tc.tile_pool(name="wpool", bufs=1))
    ptp = ctx.enter_context(tc.tile_pool(name="ptp", bufs=7))
    ostage = ctx.enter_context(tc.tile_pool(name="ostage", bufs=2))
    ostg = ctx.enter_context(tc.tile_pool(name="ostg", bufs=2))
    dscr = ctx.enter_context(tc.tile_pool(name="dscr", bufs=2, space="DRAM"))
    ps_small = ctx.enter_context(tc.tile_pool(name="ps_small", bufs=2, space="PSUM"))
    ps_st = ctx.enter_context(tc.tile_pool(name="ps_st", bufs=2, space="PSUM"))
    ps_oaug = ctx.enter_context(tc.tile_pool(name="ps_oaug", bufs=2, space="PSUM"))

    # ---- resident input tiles (block-major, matching the DRAM layout) ----
    xTs = wpool.tile([128, NS, NK, 512], BF16, name="xTs")
    cTs = wpool.tile([128, NS, NK, 512], BF16, name="cTs")
    wq_s = wpool.tile([128, NK, 128], BF16, name="wq_s")
    wk_s = wpool.tile([128, NK, 128], BF16, name="wk_s")
    wv_s = wpool.tile([128, NK, 128], BF16, name="wv_s")
    wo_s = wpool.tile([128, D], BF16, name="wo_s")

    qT = wpool.tile([128, SEQ], BF16, name="qT")
    kT = wpool.tile([128, SEQ], BF16, name="kT")
    vT = wpool.tile([128, SEQ], BF16, name="vT")
    OT = wpool.tile([128, SEQ], BF16, name="OT")
    # V transposed per (m-chunk, head): h0 features at cols 0..63 with a
    # ones column at 64, h1 at 65..128 with ones at 129, so each head's
    # [128, 65] PV stationary accumulates its softmax denominator in
    # output row 64.
    Vall = wpool.tile([128, NM, VP], BF16, name="Vall")
    nc.vector.memset(Vall[:, :, DH : DH + 1], 1.0)
    nc.vector.memset(Vall[:, :, 2 * DH + 1 : 2 * DH + 2], 1.0)
    ident = wpool.tile([128, DH], BF16, name="ident")
    make_identity(nc, ident[0:DH, :])
    make_identity(nc, ident[DH:128, :])

    def act_block(src_d, dst, j, ring, pieces=1):
        """DMA s/g-block j of an activation into SBUF (contiguous)."""
        src = src_d.ap()[j * 128 : (j + 1) * 128, :].rearrange(
            "p (k c) -> p k c", c=512
        )
        step = NK // pieces
        for q in range(pieces):
            k0 = q * step
            ring.dma_start(
                out=dst[:, j, k0 : k0 + step, :],
                in_=src[:, k0 : k0 + step, :],
            )

    # ring schedule (each ring is a FIFO; ~110 GB/s each when all three
    # stream, HBM-bound in aggregate).  The critical chain to the first
    # exp is wq+xT0 / wk+cT0 / wv; later context blocks land on the
    # scalar ring, later x blocks and all outputs on the gpsimd ring.
    nc.gpsimd.dma_start(out=wk_s, in_=wk_d.ap().rearrange("p (k c) -> p k c", k=NK))
    nc.gpsimd.dma_start(out=wq_s, in_=wq_d.ap().rearrange("p (k c) -> p k c", k=NK))
    act_block(cT_d, cTs, 0, nc.sync, pieces=4)
    act_block(cT_d, cTs, 1, nc.sync, pieces=2)
    act_block(xT_d, xTs, 0, nc.scalar, pieces=4)
    act_block(cT_d, cTs, 2, nc.scalar, pieces=2)
    act_block(cT_d, cTs, 3, nc.scalar)
    nc.gpsimd.dma_start(out=wv_s, in_=wv_d.ap().rearrange("p (k c) -> p k c", k=NK))
    act_block(xT_d, xTs, 1, nc.gpsimd)
    act_block(xT_d, xTs, 2, nc.gpsimd)
    act_block(xT_d, xTs, 3, nc.gpsimd)
    nc.gpsimd.dma_start(out=wo_s, in_=wo_d.ap())

    def proj(dst, w_s, srcs, s):
        """dst[:, s*512:(s+1)*512] = w_s.T @ srcs chunks (contract D)."""
        acc = ps_small.tile([128, 512], F32, name="proj_acc", tag="small")
        for k in range(NK):
            nc.tensor.matmul(
                acc, w_s[:, k, :], srcs[:, s, k, :],
                start=(k == 0), stop=(k == NK - 1),
            )
        nc.vector.tensor_copy(out=dst[:, s * 512 : (s + 1) * 512], in_=acc)
        return acc

    def kv_proj(g):
        kacc = ps_small.tile([128, 512], F32, name="kacc", tag="small")
        vacc = ps_small.tile([128, 512], F32, name="vacc", tag="small")
        for k in range(NK):
            chunk = cTs[:, g, k, :]
            nc.tensor.matmul(kacc, wk_s[:, k, :], chunk,
                             start=(k == 0), stop=(k == NK - 1))
            nc.tensor.matmul(vacc, wv_s[:, k, :], chunk,
                             start=(k == 0), stop=(k == NK - 1))
        nc.vector.tensor_copy(out=kT[:, g * 512 : (g + 1) * 512], in_=kacc)
        nc.vector.tensor_copy(out=vT[:, g * 512 : (g + 1) * 512], in_=vacc)

    def v_transpose(g):
        """PE-transpose vT m-chunks into Vall."""
        for mc in range(4 * g, 4 * g + 4):
            for h in range(2):
                tp = ps_small.tile([128, 512], BF16, name="tp", tag="small")
                nc.tensor.transpose(
                    tp[:, 0:DH],
                    vT[h * DH : (h + 1) * DH, mc * 128 : (mc + 1) * 128],
                    ident[h * DH : (h + 1) * DH, :],
                )
                nc.vector.tensor_copy(
                    out=Vall[:, mc, h * 65 : h * 65 + DH], in_=tp[:, 0:DH]
                )

    def st_mm(s, mc):
        n0, n1 = s * 512, (s + 1) * 512
        m0, m1 = mc * 128, (mc + 1) * 128
        st = ps_st.tile([128, 1024], F32, name="st", tag="st")
        nc.tensor.matmul(
            st[:, 0:512], kT[0:DH, m0:m1], qT[0:DH, n0:n1],
            start=True, stop=True, tile_position=(0, 0),
        )
        nc.tensor.matmul(
            st[:, 512:1024], kT[DH:128, m0:m1], qT[DH:128, n0:n1],
            start=True, stop=True, tile_position=(64, 0),
        )
        return st

    def exp_pv(st, oaug, mc):
        pt = ptp.tile([128, 1024], BF16, name="pt", tag="pt")
        nc.scalar.activation(out=pt, in_=st, func=EXPF, bias=0.0, scale=SCALE)
        nc.tensor.matmul(
            oaug[0], Vall[:, mc, 0 : DH + 1], pt[:, 0:512],
            start=(mc == 0), stop=(mc == NM - 1),
        )
        nc.tensor.matmul(
            oaug[1], Vall[:, mc, 65 : 65 + DH + 1], pt[:, 512:1024],
            start=(mc == 0), stop=(mc == NM - 1),
        )

    def mk_oaug(s):
        return [
            ps_oaug.tile([DH + 1, 512], F32, name=f"oaug{s}_{h}", tag="oaug")
            for h in range(2)
        ]

    def fin(s, oaug):
        """Normalize by softmax denominators into OT.

        Each head's denominator sits in row 64 of its accumulator (the
        ones column).  The [1, 512] denominator row is repartitioned to
        [128, 4] and back via DMA so the reciprocal runs on all DVE
        lanes.
        """
        n0, n1 = s * 512, (s + 1) * 512
        for h in range(2):
            oaug_sb = ostage.tile([DH + 1, 512], F32, name="oaug_sb",
                                  tag=f"oaug_sb{h}")
            nc.vector.tensor_copy(out=oaug_sb, in_=oaug[h])
            orows = slice(0, DH)
            # h1's chain on the scalar ring so the two heads' repartition
            # hops run concurrently (ring slices do not block the ACT
            # engine's compute queue); sync ring carries h0 only.
            ring = nc.sync if h == 0 else nc.scalar
            den_p = ostage.tile([128, 4], F32, name="den_p", tag=f"den_p{h}")
            ring.dma_start(out=den_p, in_=oaug_sb[DH : DH + 1, :])
            rec_p = ostage.tile([128, 4], F32, name="rec_p", tag=f"rec_p{h}")
            nc.vector.reciprocal(out=rec_p, in_=den_p)
            scr2 = dscr.tile([128, 4], F32, name="scr2", tag=f"scr2{h}")
            ring.dma_start(out=scr2, in_=rec_p)
            recip_rep = ostage.tile([DH, 512], F32, name="recip_rep",
                                    tag=f"recip_rep{h}")
            ring.dma_start(
                out=recip_rep,
                in_=scr2.rearrange("p f -> (p f)").partition_broadcast(DH),
            )
            nc.vector.tensor_mul(
                out=OT[h * DH : (h + 1) * DH, n0:n1],
                in0=oaug_sb[orows, :],
                in1=recip_rep,
            )

    def outproj_nt(s, nt, ost):
        """One n-tile (128 rows) of the out projection into staging."""
        for half in range(2):
            c0, c1 = half * 512, (half + 1) * 512
            ops = ps_small.tile([128, 512], F32, name="ops", tag="small")
            nc.tensor.matmul(
                ops, OT[:, (s * 4 + nt) * 128 : (s * 4 + nt + 1) * 128],
                wo_s[:, c0:c1], start=True, stop=True,
            )
            nc.vector.tensor_copy(out=ost[:, nt, c0:c1], in_=ops)

    def out_dma(s, ost):
        """Write staging to DRAM on the gpsimd ring only -- the sync and
        scalar rings carry the latency-sensitive fin chains."""
        dst = out_d.ap()[s * 512 : (s + 1) * 512, :].rearrange(
            "(nt p) d -> p nt d", p=128
        )
        nc.gpsimd.dma_start(out=dst[:, 0:2, :], in_=ost[:, 0:2, :])
        nc.gpsimd.dma_start(out=dst[:, 2:4, :], in_=ost[:, 2:4, :])

    # ---- schedule ----
    # The exp stream on ScalarE is the pacing engine.  Tiles are emitted
    # in a global order that bridges the late cT3 arrival (s0 tiles
    # 12-15 are deferred past s1 tiles 0-3), with St(next) always ahead
    # of PV(cur) so the PE never sits between ScalarE's exp ticks.
    # tile_set_cur_wait encodes estimated real-time pacing so the Tile
    # scheduler cannot hoist filler matmuls ahead of the St/exp chain.
    w = tc.tile_set_cur_wait
    PACE = 0.0011
    T0 = 0.014

    order = (
        [(0, mc) for mc in range(12)]
        + [(1, mc) for mc in range(4)]
        + [(0, mc) for mc in range(12, 16)]
        + [(1, mc) for mc in range(4, 16)]
        + [(2, mc) for mc in range(16)]
        + [(3, mc) for mc in range(16)]
    )
    s_done = {0: 19, 1: 31, 2: 47, 3: 63}  # stream pos of each s's last tile

    w(0.008)
    kv_proj(0)
    v_transpose(0)
    w(0.0095)
    proj(qT, wq_s, xTs, 0)

    oaugs = {0: mk_oaug(0), 1: mk_oaug(1)}
    st_tiles = {}
    osts = {}
    qaccs = {}

    def ensure_st(p):
        if p >= len(order):
            return
        s, mc = order[p]
        if (s, mc) not in st_tiles:
            st_tiles[(s, mc)] = st_mm(s, mc)

    def qproj_part(sq, half):
        """Half of qT chunk sq's projection (4 contraction chunks)."""
        if half == 0:
            qaccs[sq] = ps_small.tile([128, 512], F32, name="qacc",
                                      tag="small")
        for k in range(half * 4, half * 4 + 4):
            nc.tensor.matmul(
                qaccs[sq], wq_s[:, k, :], xTs[:, sq, k, :],
                start=(k == 0), stop=(k == NK - 1),
            )
        if half == 1:
            nc.vector.tensor_copy(
                out=qT[:, sq * 512 : (sq + 1) * 512], in_=qaccs.pop(sq)
            )

    # filler windows: qproj bursts and outproj windows placed so their
    # ps_small slots never overlap a kv burst or each other.
    QPROJ = {1: (1, 2), 2: (24, 25), 3: (40, 41)}
    # kv(g) emitted at end of position p: must precede ensure_st of the
    # first tile reading it -- (0,4)@p3, (0,8)@p7, (0,12)@p15.
    KV = {2: 1, 6: 2, 13: 3}
    OUTP = {0: 27, 1: 35, 2: 51}  # first outproj position per chunk

    ensure_st(0)
    for p in range(len(order)):
        s, mc = order[p]
        w(T0 + p * PACE)
        exp_pv(st_tiles.pop((s, mc)), oaugs[s], mc)
        # interleaved filler work
        for sq, (p0, p1) in QPROJ.items():
            if p == p0:
                qproj_part(sq, 0)
            elif p == p1:
                qproj_part(sq, 1)
        for sf, pos in s_done.items():
            if p == pos and sf < NS - 1:
                fin(sf, oaugs.pop(sf))
                if sf + 2 < NS:
                    oaugs[sf + 2] = mk_oaug(sf + 2)
        for sf, p0 in OUTP.items():
            if p == p0 - 1:
                osts[sf] = ostg.tile([128, 4, 1024], F32, name="ost",
                                     tag="ost")
            elif p0 <= p < p0 + 4:
                outproj_nt(sf, p - p0, osts[sf])
                if p == p0 + 3:
                    out_dma(sf, osts.pop(sf))
        ensure_st(p + 1)
        # kv chunks chase the cT stream; emitted after St(p+1) so the
        # burst never delays the next exp tick.
        if p in KV:
            g = KV[p]
            kv_proj(g)
            v_transpose(g)

    # tail: last chunk finalized with per-ntile output DMAs on 3 rings.
    # outproj uses the (now free) ps_st pool so two n-tiles of matmuls
    # stay in flight per evacuation copy instead of MM/COPY ping-pong.
    w(T0 + 64 * PACE)
    fin(NS - 1, oaugs.pop(NS - 1))
    ost = ostg.tile([128, 4, 1024], F32, name="ost", tag="ost")
    dst3 = out_d.ap()[(NS - 1) * 512 : NS * 512, :].rearrange(
        "(nt p) d -> p nt d", p=128
    )
    for nt in range(4):
        big = ps_st.tile([128, 1024], F32, name="opb", tag="st")
        for half in range(2):
            c0, c1 = half * 512, (half + 1) * 512
            nc.tensor.matmul(
                big[:, c0:c1],
                OT[:, ((NS - 1) * 4 + nt) * 128 : ((NS - 1) * 4 + nt + 1) * 128],
                wo_s[:, c0:c1], start=True, stop=True,
            )
        nc.vector.tensor_copy(out=ost[:, nt, :], in_=big)
        if nt == 1:
            nc.gpsimd.dma_start(out=dst3[:, 0:2, :], in_=ost[:, 0:2, :])
        elif nt == 2:
            nc.sync.dma_start(out=dst3[:, 2:3, :], in_=ost[:, 2:3, :])
        elif nt == 3:
            nc.scalar.dma_start(out=dst3[:, 3:4, :], in_=ost[:, 3:4, :])

    if dbg is not None:
        dumps = {"dbg_qT": qT, "dbg_kT": kT, "dbg_vT": vT, "dbg_OT": OT,
                 "dbg_Vall": Vall}
        for name, t in dumps.items():
            if name in dbg:
                nc.sync.dma_start(
                    out=dbg[name].ap(),
                    in_=t.rearrange("p a b -> p (a b)")
                    if len(t.shape) == 3 else t[:, :],
                )

    ctx.close()


_NC = None


def _get_nc():
    global _NC
    if _NC is None:
        _NC = build_nc()
    return _NC


def _np_bf16():
    import ml_dtypes

    return ml_dtypes.bfloat16


def _swizzle(w):
    """[1024, 128] -> [128, 8*128]: chunk k of the contraction dim lands in
    column block k, so the device DMA is fully contiguous."""
    return np.ascontiguousarray(
        np.asarray(w, np.float32).reshape(NK, 128, F).transpose(1, 0, 2)
        .reshape(128, NK * F).astype(_np_bf16())
    )


def shard_inputs(x, context, Wq, Wk, Wv, Wo, bo):
    x = np.asarray(x, np.float32)
    context = np.asarray(context, np.float32)
    Wq = np.asarray(Wq, np.float32)
    Wk = np.asarray(Wk, np.float32)
    Wv = np.asarray(Wv, np.float32)
    Wo = np.asarray(Wo, np.float32)

    at = _np_bf16()

    def blockify(a):
        """[n, D] -> [NS*128, NK*512]: seq block j as [128, NK*512] with
        contiguous per-partition runs (partition p = d % 128, chunk k)."""
        t = a.T.reshape(NK, 128, NS, 512)  # [k, p, j, c]
        return np.ascontiguousarray(
            t.transpose(2, 1, 0, 3).reshape(NS * 128, NK * 512)
        ).astype(at)

    xT = [blockify(x[b]) for b in range(x.shape[0])]
    cT = [blockify(context[b]) for b in range(context.shape[0])]
    in_maps = []
    for c in range(8):
        b, hp = divmod(c, 4)
        f0 = hp * F
        in_maps.append(
            {
                "xT": xT[b],
                "cT": cT[b],
                "wq": _swizzle(Wq[:, f0 : f0 + F]),
                "wk": _swizzle(Wk[:, f0 : f0 + F]),
                "wv": _swizzle(Wv[:, f0 : f0 + F]),
                "wo": np.ascontiguousarray(Wo[f0 : f0 + F, :]).astype(at),
            }
        )
    return in_maps


def kernel(x, context, Wq, Wk, Wv, Wo, bo):
    from concourse.bass_utils import run_bass_kernel_spmd

    in_maps = shard_inputs(x, context, Wq, Wk, Wv, Wo, bo)
    nc = _get_nc()
    res = run_bass_kernel_spmd(nc, in_maps, list(range(8)))
    out = np.zeros((2, SEQ, D), np.float32)
    for c in range(8):
        out[c // 4] += res.results[c]["out_p"]
    out += np.asarray(bo, np.float32).reshape(1, 1, D)
    return out


# revision 36
# speedup vs baseline: 1.0928x; 1.0928x over previous
"""Cross-attention kernel for Trainium2, 8 NeuronCores.

Sharding (data + head parallel, per the problem's sharding hint):
  core c in 0..7 -> batch b = c // 4, head-pair hp = c % 4.
  Each core computes attention for its batch with 2 of the 8 heads
  (a 128-wide slice of the 512 hidden features), then the partial
  out-projection  attn_out_slice @ Wo[slice, :].  The host sums the 4
  partials per batch (the "all-reduce") and adds bo.

Device-side dataflow per core (all matmuls bf16, feature-major):
  All inputs land in SBUF via a few ~1MB DMAs spread over the three
  DMA rings (sync/SP HWDGE, scalar/ACT HWDGE, gpsimd SWDGE) so no ring
  serializes the kernel.
  qT[128, N] = Wq_sl.T @ xT          (contraction over D=1024 in 8 chunks)
  kT[128, M] = Wk_sl.T @ cT ; vT likewise
  Vall[m, 130] = DMA-xbar transpose of vT per m-chunk, with ones
      columns at 0 and 129: PV stationary for h0 = cols 0:65 (den in
      output row 0), for h1 = cols 65:130 (den in output row 64).
  per n-chunk s (512 cols), per m-chunk mc (128 rows), software-
  pipelined so ScalarE (exp) never idles:
     St[m 128, 1024] = [kT_h0_mc.T @ qT_h0_s | kT_h1_mc.T @ qT_h1_s]
         (concurrent matmuls on PE row-groups 0-63 / 64-127)
     Pt = exp(St * 1/8)               (ScalarE -- the bottleneck engine)
     Oaug_h[65, 512] += Vall_h_mc.T @ Pt_h               (PSUM accum)
  fin: denominators from the ones rows; reciprocal repartitioned via a
  DRAM bounce; OT[h*64:, s] = O_h / den_h.
  outproj: out[n 128, 1024] = OT_ntile.T @ Wo_sl, staged in SBUF,
  written back as two 1MB DMAs per n-chunk (bias bo added on host).
"""

import numpy as np

import concourse.bass as bass
import concourse.tile as tile
from concourse import bacc, mybir
from concourse.masks import make_identity

F32 = mybir.dt.float32
BF16 = mybir.dt.bfloat16

D = 1024      # model dim (contraction for projections)
SEQ = 2048    # n == m
F = 128       # features per core (2 heads x 64)
DH = 64       # head dim
NS = SEQ // 512   # 4 n-chunks of 512
NK = D // 128     # 8 contraction chunks
NM = SEQ // 128   # 16 m-chunks of 128
VP = 132          # Vall row stride (130 used + 2 pad)
SCALE = DH ** -0.5
EXPF = mybir.ActivationFunctionType.Exp


def build_nc():
    nc = bacc.Bacc("TRN2", target_bir_lowering=False, debug=False)

    # x/ctx arrive host-blocked: block j (512 seq cols) is [128, NK*512]
    # with fully contiguous per-partition runs, so each block DMA needs
    # only 128 8KB-contiguous descriptors.
    xT_d = nc.dram_tensor("xT", [NS * 128, NK * 512], BF16, kind="ExternalInput")
    cT_d = nc.dram_tensor("cT", [NS * 128, NK * 512], BF16, kind="ExternalInput")
    # wq/wk/wv arrive pre-swizzled by the host: [128, NK*128] where
    # column block k holds W[k*128:(k+1)*128, :].T-chunk laid contiguously.
    wq_d = nc.dram_tensor("wq", [128, NK * 128], BF16, kind="ExternalInput")
    wk_d = nc.dram_tensor("wk", [128, NK * 128], BF16, kind="ExternalInput")
    wv_d = nc.dram_tensor("wv", [128, NK * 128], BF16, kind="ExternalInput")
    wo_d = nc.dram_tensor("wo", [F, D], BF16, kind="ExternalInput")
    out_d = nc.dram_tensor("out_p", [SEQ, D], F32, kind="ExternalOutput")

    with tile.TileContext(nc) as tc:
        _emit(tc, nc, xT_d, cT_d, wq_d, wk_d, wv_d, wo_d, out_d)
    nc.compile()
    return nc


def _emit(tc, nc, xT_d, cT_d, wq_d, wk_d, wv_d, wo_d, out_d, dbg=None):
    from contextlib import ExitStack

    ctx = ExitStack()
    wpool = ctx.enter_context(tc.tile_pool(name="wpool", bufs=1))
    ptp = ctx.enter_context(tc.tile_pool(name="ptp", bufs=7))
    ostage = ctx.enter_context(tc.tile_pool(name="ostage", bufs=2))
    ostg = ctx.enter_context(tc.tile_pool(name="ostg", bufs=2))
    dscr = ctx.enter_context(tc.tile_pool(name="dscr", bufs=2, space="DRAM"))
    ps_small = ctx.enter_context(tc.tile_pool(name="ps_small", bufs=2, space="PSUM"))
    ps_st = ctx.enter_context(tc.tile_pool(name="ps_st", bufs=2, space="PSUM"))
    ps_oaug = ctx.enter_context(tc.tile_pool(name="ps_oaug", bufs=2, space="PSUM"))

    # ---- resident input tiles (block-major, matching the DRAM layout) ----
    xTs = wpool.tile([128, NS, NK, 512], BF16, name="xTs")
    cTs = wpool.tile([128, NS, NK, 512], BF16, name="cTs")
    wq_s = wpool.tile([128, NK, 128], BF16, name="wq_s")
    wk_s = wpool.tile([128, NK, 128], BF16, name="wk_s")
    wv_s = wpool.tile([128, NK, 128], BF16, name="wv_s")
    wo_s = wpool.tile([128, D], BF16, name="wo_s")

    qT = wpool.tile([128, SEQ], BF16, name="qT")
    kT = wpool.tile([128, SEQ], BF16, name="kT")
    vT = wpool.tile([128, SEQ], BF16, name="vT")
    OT = wpool.tile([128, SEQ], BF16, name="OT")
    # V transposed per (m-chunk, head): h0 features at cols 0..63 with a
    # ones column at 64, h1 at 65..128 with ones at 129, so each head's
    # [128, 65] PV stationary accumulates its softmax denominator in
    # output row 64.
    Vall = wpool.tile([128, NM, VP], BF16, name="Vall")
    nc.vector.memset(Vall[:, :, DH : DH + 1], 1.0)
    nc.vector.memset(Vall[:, :, 2 * DH + 1 : 2 * DH + 2], 1.0)
    ident = wpool.tile([128, DH], BF16, name="ident")
    make_identity(nc, ident[0:DH, :])
    make_identity(nc, ident[DH:128, :])

    def act_block(src_d, dst, j, ring, pieces=1):
        """DMA s/g-block j of an activation into SBUF (contiguous)."""
        src = src_d.ap()[j * 128 : (j + 1) * 128, :].rearrange(
            "p (k c) -> p k c", c=512
        )
        step = NK // pieces
        for q in range(pieces):
            k0 = q * step
            ring.dma_start(
                out=dst[:, j, k0 : k0 + step, :],
                in_=src[:, k0 : k0 + step, :],
            )

    # ring schedule (each ring is a FIFO; ~110 GB/s each when all three
    # stream, HBM-bound in aggregate).  The critical chain to the first
    # exp is wq+xT0 / wk+cT0 / wv; later context blocks land on the
    # scalar ring, later x blocks and all outputs on the gpsimd ring.
    nc.sync.dma_start(out=wk_s, in_=wk_d.ap().rearrange("p (k c) -> p k c", k=NK))
    act_block(cT_d, cTs, 0, nc.sync, pieces=4)
    act_block(cT_d, cTs, 1, nc.sync, pieces=2)
    nc.scalar.dma_start(out=wq_s, in_=wq_d.ap().rearrange("p (k c) -> p k c", k=NK))
    act_block(xT_d, xTs, 0, nc.scalar, pieces=4)
    act_block(cT_d, cTs, 2, nc.scalar, pieces=2)
    act_block(cT_d, cTs, 3, nc.scalar)
    nc.gpsimd.dma_start(out=wv_s, in_=wv_d.ap().rearrange("p (k c) -> p k c", k=NK))
    act_block(xT_d, xTs, 1, nc.gpsimd)
    act_block(xT_d, xTs, 2, nc.gpsimd)
    act_block(xT_d, xTs, 3, nc.gpsimd)
    nc.gpsimd.dma_start(out=wo_s, in_=wo_d.ap())

    def proj(dst, w_s, srcs, s):
        """dst[:, s*512:(s+1)*512] = w_s.T @ srcs chunks (contract D)."""
        acc = ps_small.tile([128, 512], F32, name="proj_acc", tag="small")
        for k in range(NK):
            nc.tensor.matmul(
                acc, w_s[:, k, :], srcs[:, s, k, :],
                start=(k == 0), stop=(k == NK - 1),
            )
        nc.vector.tensor_copy(out=dst[:, s * 512 : (s + 1) * 512], in_=acc)
        return acc

    def kv_proj(g):
        kacc = ps_small.tile([128, 512], F32, name="kacc", tag="small")
        vacc = ps_small.tile([128, 512], F32, name="vacc", tag="small")
        for k in range(NK):
            chunk = cTs[:, g, k, :]
            nc.tensor.matmul(kacc, wk_s[:, k, :], chunk,
                             start=(k == 0), stop=(k == NK - 1))
            nc.tensor.matmul(vacc, wv_s[:, k, :], chunk,
                             start=(k == 0), stop=(k == NK - 1))
        nc.vector.tensor_copy(out=kT[:, g * 512 : (g + 1) * 512], in_=kacc)
        nc.vector.tensor_copy(out=vT[:, g * 512 : (g + 1) * 512], in_=vacc)

    def v_transpose(g):
        """PE-transpose vT m-chunks into Vall."""
        for mc in range(4 * g, 4 * g + 4):
            for h in range(2):
                tp = ps_small.tile([128, 512], BF16, name="tp", tag="small")
                nc.tensor.transpose(
                    tp[:, 0:DH],
                    vT[h * DH : (h + 1) * DH, mc * 128 : (mc + 1) * 128],
                    ident[h * DH : (h + 1) * DH, :],
                )
                nc.vector.tensor_copy(
                    out=Vall[:, mc, h * 65 : h * 65 + DH], in_=tp[:, 0:DH]
                )

    def st_mm(s, mc):
        n0, n1 = s * 512, (s + 1) * 512
        m0, m1 = mc * 128, (mc + 1) * 128
        st = ps_st.tile([128, 1024], F32, name="st", tag="st")
        nc.tensor.matmul(
            st[:, 0:512], kT[0:DH, m0:m1], qT[0:DH, n0:n1],
            start=True, stop=True, tile_position=(0, 0),
        )
        nc.tensor.matmul(
            st[:, 512:1024], kT[DH:128, m0:m1], qT[DH:128, n0:n1],
            start=True, stop=True, tile_position=(64, 0),
        )
        return st

    def exp_pv(st, oaug, mc):
        pt = ptp.tile([128, 1024], BF16, name="pt", tag="pt")
        nc.scalar.activation(out=pt, in_=st, func=EXPF, bias=0.0, scale=SCALE)
        nc.tensor.matmul(
            oaug[0], Vall[:, mc, 0 : DH + 1], pt[:, 0:512],
            start=(mc == 0), stop=(mc == NM - 1),
        )
        nc.tensor.matmul(
            oaug[1], Vall[:, mc, 65 : 65 + DH + 1], pt[:, 512:1024],
            start=(mc == 0), stop=(mc == NM - 1),
        )

    def mk_oaug(s):
        return [
            ps_oaug.tile([DH + 1, 512], F32, name=f"oaug{s}_{h}", tag="oaug")
            for h in range(2)
        ]

    def fin(s, oaug):
        """Normalize by softmax denominators into OT.

        Each head's denominator sits in row 64 of its accumulator (the
        ones column).  The [1, 512] denominator row is repartitioned to
        [128, 4] and back via DMA so the reciprocal runs on all DVE
        lanes.
        """
        n0, n1 = s * 512, (s + 1) * 512
        for h in range(2):
            oaug_sb = ostage.tile([DH + 1, 512], F32, name="oaug_sb",
                                  tag=f"oaug_sb{h}")
            nc.vector.tensor_copy(out=oaug_sb, in_=oaug[h])
            orows = slice(0, DH)
            # h1's chain on a second ring so the two heads' repartition
            # hops run concurrently; scalar ring only for the tail fin
            # (ACT is idle there), gpsimd otherwise.
            if h == 0:
                ring = nc.sync
            else:
                ring = nc.scalar if s == NS - 1 else nc.gpsimd
            den_p = ostage.tile([128, 4], F32, name="den_p", tag=f"den_p{h}")
            ring.dma_start(out=den_p, in_=oaug_sb[DH : DH + 1, :])
            rec_p = ostage.tile([128, 4], F32, name="rec_p", tag=f"rec_p{h}")
            nc.vector.reciprocal(out=rec_p, in_=den_p)
            scr2 = dscr.tile([128, 4], F32, name="scr2", tag=f"scr2{h}")
            ring.dma_start(out=scr2, in_=rec_p)
            recip_rep = ostage.tile([DH, 512], F32, name="recip_rep",
                                    tag=f"recip_rep{h}")
            ring.dma_start(
                out=recip_rep,
                in_=scr2.rearrange("p f -> (p f)").partition_broadcast(DH),
            )
            nc.vector.tensor_mul(
                out=OT[h * DH : (h + 1) * DH, n0:n1],
                in0=oaug_sb[orows, :],
                in1=recip_rep,
            )

    def outproj_nt(s, nt, ost):
        """One n-tile (128 rows) of the out projection into staging."""
        for half in range(2):
            c0, c1 = half * 512, (half + 1) * 512
            ops = ps_small.tile([128, 512], F32, name="ops", tag="small")
            nc.tensor.matmul(
                ops, OT[:, (s * 4 + nt) * 128 : (s * 4 + nt + 1) * 128],
                wo_s[:, c0:c1], start=True, stop=True,
            )
            nc.vector.tensor_copy(out=ost[:, nt, c0:c1], in_=ops)

    def out_dma(s, ost):
        """Write staging to DRAM as two 1MB DMAs on idle rings."""
        dst = out_d.ap()[s * 512 : (s + 1) * 512, :].rearrange(
            "(nt p) d -> p nt d", p=128
        )
        nc.gpsimd.dma_start(out=dst[:, 0:2, :], in_=ost[:, 0:2, :])
        ring = nc.sync if s < NS - 1 else nc.scalar
        ring.dma_start(out=dst[:, 2:4, :], in_=ost[:, 2:4, :])

    # ---- schedule ----
    # The exp stream on ScalarE is the pacing engine.  Tiles are emitted
    # in a global order that bridges the late cT3 arrival (s0 tiles
    # 12-15 are deferred past s1 tiles 0-3), with St(next) always ahead
    # of PV(cur) so the PE never sits between ScalarE's exp ticks.
    # tile_set_cur_wait encodes estimated real-time pacing so the Tile
    # scheduler cannot hoist filler matmuls ahead of the St/exp chain.
    w = tc.tile_set_cur_wait
    PACE = 0.0011
    T0 = 0.014

    order = (
        [(0, mc) for mc in range(12)]
        + [(1, mc) for mc in range(4)]
        + [(0, mc) for mc in range(12, 16)]
        + [(1, mc) for mc in range(4, 16)]
        + [(2, mc) for mc in range(16)]
        + [(3, mc) for mc in range(16)]
    )
    s_done = {0: 19, 1: 31, 2: 47, 3: 63}  # stream pos of each s's last tile

    w(0.008)
    kv_proj(0)
    v_transpose(0)
    w(0.0095)
    proj(qT, wq_s, xTs, 0)

    oaugs = {0: mk_oaug(0), 1: mk_oaug(1)}
    st_tiles = {}
    osts = {}
    qaccs = {}

    def ensure_st(p):
        if p >= len(order):
            return
        s, mc = order[p]
        if (s, mc) not in st_tiles:
            st_tiles[(s, mc)] = st_mm(s, mc)

    def qproj_part(sq, half):
        """Half of qT chunk sq's projection (4 contraction chunks)."""
        if half == 0:
            qaccs[sq] = ps_small.tile([128, 512], F32, name="qacc",
                                      tag="small")
        for k in range(half * 4, half * 4 + 4):
            nc.tensor.matmul(
                qaccs[sq], wq_s[:, k, :], xTs[:, sq, k, :],
                start=(k == 0), stop=(k == NK - 1),
            )
        if half == 1:
            nc.vector.tensor_copy(
                out=qT[:, sq * 512 : (sq + 1) * 512], in_=qaccs.pop(sq)
            )

    # filler windows: qproj bursts and outproj windows placed so their
    # ps_small slots never overlap a kv burst or each other.
    QPROJ = {1: (1, 2), 2: (24, 25), 3: (40, 41)}
    # kv(g) emitted at end of position p: must precede ensure_st of the
    # first tile reading it -- (0,4)@p3, (0,8)@p7, (0,12)@p15.
    KV = {2: 1, 6: 2, 13: 3}
    OUTP = {0: 27, 1: 35, 2: 51}  # first outproj position per chunk

    ensure_st(0)
    for p in range(len(order)):
        s, mc = order[p]
        w(T0 + p * PACE)
        exp_pv(st_tiles.pop((s, mc)), oaugs[s], mc)
        # interleaved filler work
        for sq, (p0, p1) in QPROJ.items():
            if p == p0:
                qproj_part(sq, 0)
            elif p == p1:
                qproj_part(sq, 1)
        for sf, pos in s_done.items():
            if p == pos and sf < NS - 1:
                fin(sf, oaugs.pop(sf))
                if sf + 2 < NS:
                    oaugs[sf + 2] = mk_oaug(sf + 2)
        for sf, p0 in OUTP.items():
            if p == p0 - 1:
                osts[sf] = ostg.tile([128, 4, 1024], F32, name="ost",
                                     tag="ost")
            elif p0 <= p < p0 + 4:
                outproj_nt(sf, p - p0, osts[sf])
                if p == p0 + 3:
                    out_dma(sf, osts.pop(sf))
        ensure_st(p + 1)
        # kv chunks chase the cT stream; emitted after St(p+1) so the
        # burst never delays the next exp tick.
        if p in KV:
            g = KV[p]
            kv_proj(g)
            v_transpose(g)

    # tail: last chunk finalized with per-ntile output DMAs on 3 rings.
    # outproj uses the (now free) ps_st pool so two n-tiles of matmuls
    # stay in flight per evacuation copy instead of MM/COPY ping-pong.
    w(T0 + 64 * PACE)
    fin(NS - 1, oaugs.pop(NS - 1))
    # keep the PE's HAM clock warm through the fin DMA-chain window
    # (~10us of otherwise-idle PE) so the tail out-projection runs at
    # 2.4 GHz instead of re-throttled 1.2 GHz.
    warm = ps_st.tile([128, 1024], F32, name="warm", tag="st")
    for i in range(14):
        nc.tensor.matmul(
            warm[:, 0:512], wq_s[:, i % NK, :], xTs[:, 0, i % NK, :],
            start=True, stop=True,
        )
    ost = ostg.tile([128, 4, 1024], F32, name="ost", tag="ost")
    dst3 = out_d.ap()[(NS - 1) * 512 : NS * 512, :].rearrange(
        "(nt p) d -> p nt d", p=128
    )
    for nt in range(4):
        big = ps_st.tile([128, 1024], F32, name="opb", tag="st")
        for half in range(2):
            c0, c1 = half * 512, (half + 1) * 512
            nc.tensor.matmul(
                big[:, c0:c1],
                OT[:, ((NS - 1) * 4 + nt) * 128 : ((NS - 1) * 4 + nt + 1) * 128],
                wo_s[:, c0:c1], start=True, stop=True,
            )
        nc.vector.tensor_copy(out=ost[:, nt, :], in_=big)
        if nt == 1:
            nc.gpsimd.dma_start(out=dst3[:, 0:2, :], in_=ost[:, 0:2, :])
        elif nt == 2:
            nc.sync.dma_start(out=dst3[:, 2:3, :], in_=ost[:, 2:3, :])
        elif nt == 3:
            nc.scalar.dma_start(out=dst3[:, 3:4, :], in_=ost[:, 3:4, :])

    if dbg is not None:
        dumps = {"dbg_qT": qT, "dbg_kT": kT, "dbg_vT": vT, "dbg_OT": OT,
                 "dbg_Vall": Vall}
        for name, t in dumps.items():
            if name in dbg:
                nc.sync.dma_start(
                    out=dbg[name].ap(),
                    in_=t.rearrange("p a b -> p (a b)")
                    if len(t.shape) == 3 else t[:, :],
                )

    ctx.close()


_NC = None


def _get_nc():
    global _NC
    if _NC is None:
        _NC = build_nc()
    return _NC


def _np_bf16():
    import ml_dtypes

    return ml_dtypes.bfloat16


def _swizzle(w):
    """[1024, 128] -> [128, 8*128]: chunk k of the contraction dim lands in
    column block k, so the device DMA is fully contiguous."""
    return np.ascontiguousarray(
        np.asarray(w, np.float32).reshape(NK, 128, F).transpose(1, 0, 2)
        .reshape(128, NK * F).astype(_np_bf16())
    )


def shard_inputs(x, context, Wq, Wk, Wv, Wo, bo):
    x = np.asarray(x, np.float32)
    context = np.asarray(context, np.float32)
    Wq = np.asarray(Wq, np.float32)
    Wk = np.asarray(Wk, np.float32)
    Wv = np.asarray(Wv, np.float32)
    Wo = np.asarray(Wo, np.float32)

    at = _np_bf16()

    def blockify(a):
        """[n, D] -> [NS*128, NK*512]: seq block j as [128, NK*512] with
        contiguous per-partition runs (partition p = d % 128, chunk k)."""
        t = a.T.reshape(NK, 128, NS, 512)  # [k, p, j, c]
        return np.ascontiguousarray(
            t.transpose(2, 1, 0, 3).reshape(NS * 128, NK * 512)
        ).astype(at)

    xT = [blockify(x[b]) for b in range(x.shape[0])]
    cT = [blockify(context[b]) for b in range(context.shape[0])]
    in_maps = []
    for c in range(8):
        b, hp = divmod(c, 4)
        f0 = hp * F
        in_maps.append(
            {
                "xT": xT[b],
                "cT": cT[b],
                "wq": _swizzle(Wq[:, f0 : f0 + F]),
                "wk": _swizzle(Wk[:, f0 : f0 + F]),
                "wv": _swizzle(Wv[:, f0 : f0 + F]),
                "wo": np.ascontiguousarray(Wo[f0 : f0 + F, :]).astype(at),
            }
        )
    return in_maps


def kernel(x, context, Wq, Wk, Wv, Wo, bo):
    from concourse.bass_utils import run_bass_kernel_spmd

    in_maps = shard_inputs(x, context, Wq, Wk, Wv, Wo, bo)
    nc = _get_nc()
    res = run_bass_kernel_spmd(nc, in_maps, list(range(8)))
    out = np.zeros((2, SEQ, D), np.float32)
    for c in range(8):
        out[c // 4] += res.results[c]["out_p"]
    out += np.asarray(bo, np.float32).reshape(1, 1, D)
    return out
